# revision 1
# baseline (speedup 1.0000x reference)
"""TRN2 Bass kernel for nn_KnnModule (retrieval_knn).

Strategy (sharded over the 8 NeuronCores):
  - train set (100000 x 1024) is padded to 102400 rows and split into 8
    chunks of 12800; each core computes sims = features @ chunk.T
    (2048 x 12800) on the PE (float32r single-pass, 1 cycle/row), and for
    every 512-wide tile of the chunk extracts the top-8 values + indices
    per row with the DVE InstMax/InstMaxIndex top-8 primitives
    (25 tiles -> 200 candidates per row per core).
  - host merges the 8 x 200 per-row candidate lists, takes the top-48 by
    approximate value, recomputes their sims exactly in fp32 (48 dot
    products per row ~ 0.03% of the device FLOPs), and reproduces the
    reference softmax voting for k in (10, 20, 100, 200).

  Why this is exact: with T=0.07 and sims ~ N(0, 37^2), a candidate's
  fp32 softmax weight is exactly 0.0 unless its sim is within ~7.3 of the
  row max. On this distribution there are at most ~15 such candidates per
  row and at most 2 per 512-wide tile (4x safety margin vs the top-8
  extraction), and float32r's max matmul error (1.8e-2) is negligible vs
  the 7.3 window. Rows that get anywhere near these margins are detected
  and recomputed exactly on the host; on this data the triggers never
  fire.
"""

import numpy as np

KS = (10, 20, 100, 200)
T = 0.07
NUM_CLASSES = 1000
B, N, D = 2048, 100000, 1024
NCORES = 8
NCHUNK = 12800  # per-core padded chunk (12500 real + 300 zero pad)
TILE_N = 512
NT = NCHUNK // TILE_N  # 25 tiles -> 200 candidate slots per core
P = 128
KEXACT = 48  # candidates per row exactly rescored on host

_NC_CACHE = {}


def _build_bass():
    import concourse.bacc as bacc
    import concourse.mybir as mybir
    import concourse.tile as tile

    mm_dtype = mybir.dt.float32r
    KO = D // P
    MB = B // P

    nc = bacc.Bacc(
        "TRN2",
        target_bir_lowering=False,
        debug=False,
        enable_asserts=False,
    )
    featT = nc.dram_tensor("featT", (D, B), mm_dtype, kind="ExternalInput")
    trainT = nc.dram_tensor("trainT", (D, NCHUNK), mm_dtype, kind="ExternalInput")
    out_val = nc.dram_tensor("t8val", (B, NT * 8), mybir.dt.float32, kind="ExternalOutput")
    out_idx = nc.dram_tensor("t8idx", (B, NT * 8), mybir.dt.uint16, kind="ExternalOutput")

    featT_ap = featT.ap().rearrange("(ko p) b -> p ko b", p=P)
    trainT_ap = trainT.ap().rearrange("(ko p) n -> p ko n", p=P)

    with tile.TileContext(nc) as tc:
        with (
            tc.tile_pool(name="const", bufs=1) as cpool,
            tc.tile_pool(name="stream", bufs=2) as spool,
            tc.tile_pool(name="acc", bufs=1) as apool,
            tc.tile_pool(name="psum", bufs=8, space="PSUM") as ppool,
        ):
            feat_sb = cpool.tile([P, KO, B], mm_dtype)
            # single DMA measured fastest: startup is HBM-BW-bound (10MB of
            # feat+train before ~30us of matmul work), so splitting this
            # into per-m chunks only delays the train tile behind 16 queued
            # transfers (measured +9us)
            nc.sync.dma_start(feat_sb, featT_ap)

            val_sb = [
                apool.tile([P, NT * 8], mybir.dt.float32, name=f"val_sb_{m}", tag=f"val{m}")
                for m in range(MB)
            ]
            idx_sb = [
                apool.tile([P, NT * 8], mybir.dt.uint16, name=f"idx_sb_{m}", tag=f"idx{m}")
                for m in range(MB)
            ]

            # train tiles processed in pairs: the same feat[ko,m] weights
            # feed two rhs tiles back-to-back, halving LDWEIGHTS pressure
            # (f32r gets no fast-weight-load; LDW=187ns vs 213ns matmul)
            t = 0
            while t < NT:
                G = 2 if t + 1 < NT else 1
                tr_sb = spool.tile(
                    [P, KO, 2 * TILE_N], mm_dtype, name="tr_sb", tag="train"
                )
                nc.sync.dma_start(
                    tr_sb[:, :, : G * TILE_N],
                    trainT_ap[:, :, t * TILE_N : (t + G) * TILE_N],
                )
                for m in range(MB):
                    pss = [
                        ppool.tile([P, TILE_N], mybir.dt.float32, name="ps", tag="ps")
                        for _ in range(G)
                    ]
                    for ko in range(KO):
                        for g in range(G):
                            nc.tensor.matmul(
                                pss[g],
                                lhsT=feat_sb[:, ko, m * P : (m + 1) * P],
                                rhs=tr_sb[:, ko, g * TILE_N : (g + 1) * TILE_N],
                                start=(ko == 0),
                                stop=(ko == KO - 1),
                            )
                    for g in range(G):
                        tt = t + g
                        vslice = val_sb[m][:, tt * 8 : (tt + 1) * 8]
                        nc.vector.max(out=vslice, in_=pss[g])
                        nc.vector.max_index(
                            out=idx_sb[m][:, tt * 8 : (tt + 1) * 8],
                            in_max=vslice,
                            in_values=pss[g],
                        )
                t += G

            ov = out_val.ap().rearrange("(mb p) c -> mb p c", p=P)
            oi = out_idx.ap().rearrange("(mb p) c -> mb p c", p=P)
            for m in range(MB):
                nc.sync.dma_start(ov[m], val_sb[m])
                nc.sync.dma_start(oi[m], idx_sb[m])

    nc.compile()
    return nc


def _get_nc():
    if "nc" not in _NC_CACHE:
        _NC_CACHE["nc"] = _build_bass()
    return _NC_CACHE["nc"]


def _vote(topv, labels):
    """Reproduce the reference's softmax voting given sorted top sims.

    topv: (B', 200) fp32 descending (padded with -inf); labels (B', 200).
    """
    Bp = topv.shape[0]
    x = (topv / np.float32(T)).astype(np.float32)
    e = np.exp(x - x[:, :1], dtype=np.float32)
    s = e.sum(axis=1, keepdims=True, dtype=np.float32)
    w = (e / s).astype(np.float32)
    rows = np.broadcast_to(np.arange(Bp)[:, None], labels.shape)
    outs = []
    for k in KS:
        p = np.zeros((Bp, NUM_CLASSES), np.float32)
        np.add.at(p, (rows[:, :k], labels[:, :k]), w[:, :k])
        outs.append(p)
    return outs


def _exact_row(F, TR, LB, b):
    s = (F[b : b + 1] @ TR.T).astype(np.float32)[0]
    o = np.argsort(-s, kind="stable")[:200]
    return _vote(s[o][None].astype(np.float32), LB[o].astype(np.int64)[None])


def _combine(F, TR, LB, vals, idxs):
    NTN = NT * 8
    slot_tile = (np.arange(NTN) // 8) * TILE_N
    gcol = (
        idxs
        + slot_tile[None, None, :]
        + (np.arange(NCORES)[:, None, None] * NCHUNK)
    )
    v = vals.transpose(1, 0, 2).reshape(B, NCORES * NTN)
    g = gcol.transpose(1, 0, 2).reshape(B, NCORES * NTN)
    v = np.where(g < N, v, -np.inf).astype(np.float32)

    # approximate top-KEXACT per row
    part = np.argpartition(-v, KEXACT, axis=1)[:, :KEXACT]
    rows = np.arange(B)[:, None]
    cand_v = v[rows, part]
    cand_g = g[rows, part]

    # exact fp32 rescoring of the candidates (0.03% of device FLOPs)
    exact = np.einsum(
        "bkd,bd->bk", TR[cand_g], F, optimize=True
    ).astype(np.float32)

    # sort by exact value desc, ties by train index asc (lax.top_k order)
    ordk = np.lexsort((cand_g, -exact.astype(np.float64)), axis=1)
    exact_s = np.take_along_axis(exact, ordk, axis=1)
    g_s = np.take_along_axis(cand_g, ordk, axis=1)

    topv = np.full((B, 200), -np.inf, np.float32)
    topv[:, :KEXACT] = exact_s
    labels = np.zeros((B, 200), np.int64)
    labels[:, :KEXACT] = LB[g_s].astype(np.int64)

    outs = _vote(topv, labels)

    # pathological-row triggers -> exact host recompute
    amax = cand_v.max(axis=1)
    # (i) too many candidates near the top (exact-significance window overflow)
    near = (cand_v >= (amax[:, None] - 8.0)).sum(axis=1)
    trig_i = near >= KEXACT - 8
    # (ii) some tile's 8th approx value near the top (dropped 9th candidate)
    v8 = vals[:, :, 7::8]  # (ncores, B, NT)
    trig_ii = v8.max(axis=(0, 2)) >= amax - 8.5
    # (iii) duplicate global col among candidates (HW tie semantics)
    ss = np.sort(cand_g, axis=1)
    trig_iii = (np.diff(ss, axis=1) == 0).any(axis=1)

    for b in np.where(trig_i | trig_ii | trig_iii)[0]:
        ob = _exact_row(F, TR, LB, b)
        for i in range(len(KS)):
            outs[i][b] = ob[i][0]

    return tuple(outs)


def kernel(features_rank, train_features, train_labels):
    from concourse.bass_utils import run_bass_kernel_spmd

    F = np.ascontiguousarray(np.asarray(features_rank, dtype=np.float32))
    TR = np.ascontiguousarray(np.asarray(train_features, dtype=np.float32))
    LB = np.asarray(train_labels)

    TRp = np.zeros((NCORES * NCHUNK, D), np.float32)
    TRp[:N] = TR
    featT = np.ascontiguousarray(F.T)

    in_maps = [
        {
            "featT": featT,
            "trainT": np.ascontiguousarray(TRp[c * NCHUNK : (c + 1) * NCHUNK].T),
        }
        for c in range(NCORES)
    ]

    nc = _get_nc()
    res = run_bass_kernel_spmd(nc, in_maps, core_ids=list(range(NCORES)))

    vals = np.stack([np.asarray(res.results[c]["t8val"]) for c in range(NCORES)])
    idxs = np.stack(
        [np.asarray(res.results[c]["t8idx"]).astype(np.int64) for c in range(NCORES)]
    )
    return _combine(F, TR, LB, vals, idxs)



# revision 3
# speedup vs baseline: 1.4485x; 1.4485x over previous
"""TRN2 Bass kernel for nn_KnnModule (retrieval_knn).

Strategy (sharded over the 8 NeuronCores):
  - train set (100000 x 1024) is padded to 102400 rows and split into 8
    chunks of 12800; each core computes sims = features @ chunk.T
    (2048 x 12800) on the PE in fp8e4 (e4m3) with MatmulPerfMode.DoubleRow
    (2 contraction halves per instruction, 0.5 cycles/row), and for
    every 512-wide tile of the chunk extracts the top-8 values + indices
    per row with the DVE InstMax/InstMaxIndex top-8 primitives
    (25 tiles -> 200 candidates per row per core).
  - host merges the 8 x 200 per-row candidate lists, takes the top-192 by
    approximate value, recomputes their sims exactly in fp32, and
    reproduces the reference softmax voting for k in (10, 20, 100, 200).

  Why this is exact: with T=0.07, a candidate's fp32 softmax weight is
  exactly 0.0 unless its sim is within ~7.3 of the row max. fp8e4
  quantization of the inputs perturbs each sim by at most ~8 (6.3 sigma
  empirically; std 1.2), so every candidate that matters sits within
  7.3 + 2*10 = ~28 of the approximate max. On this data the worst
  approximate global rank of any such candidate is 18 (vs 192 kept), and
  no 512-tile ever drops one from its top-8. Rows that get anywhere near
  these margins are detected via the margin triggers below and recomputed
  exactly on the host.
"""

import numpy as np
import ml_dtypes

KS = (10, 20, 100, 200)
T = 0.07
NUM_CLASSES = 1000
B, N, D = 2048, 100000, 1024
NCORES = 8
NCHUNK = 12800  # per-core padded chunk (12500 real + 300 zero pad)
TILE_N = 512
NT = NCHUNK // TILE_N  # 25 tiles -> 200 candidate slots per core
P = 128
KO2 = D // 256  # DoubleRow contracts 256 rows of D per matmul
KEXACT = 192  # candidates per row exactly rescored on host
MARGIN = 28.0  # fp8 selection-safety margin (7.3 softmax window + 2*~10)

_NC_CACHE = {}


def _build_bass():
    import concourse.bacc as bacc
    import concourse.mybir as mybir
    import concourse.tile as tile

    mm_dtype = mybir.dt.float8e4
    MB = B // P

    nc = bacc.Bacc(
        "TRN2",
        target_bir_lowering=False,
        debug=False,
        enable_asserts=False,
    )
    featT = nc.dram_tensor("featT", (D, B), mm_dtype, kind="ExternalInput")
    trainT = nc.dram_tensor("trainT", (D, NCHUNK), mm_dtype, kind="ExternalInput")
    out_val = nc.dram_tensor("t8val", (B, NT * 8), mybir.dt.float32, kind="ExternalOutput")
    out_idx = nc.dram_tensor("t8idx", (B, NT * 8), mybir.dt.uint16, kind="ExternalOutput")

    # DoubleRow pairing: sim contribution of D-rows d = ko2*256 + half*128 + p
    featT_ap = featT.ap().rearrange("(ko two p) b -> p ko two b", p=P, two=2)
    trainT_ap = trainT.ap().rearrange("(ko two p) n -> p ko two n", p=P, two=2)

    with tile.TileContext(nc) as tc:
        with (
            tc.tile_pool(name="const", bufs=1) as cpool,
            tc.tile_pool(name="stream", bufs=2) as spool,
            tc.tile_pool(name="acc", bufs=1) as apool,
            tc.tile_pool(name="psum", bufs=8, space="PSUM") as ppool,
        ):
            feat_sb = cpool.tile([P, KO2, 2, B], mm_dtype)
            nc.sync.dma_start(feat_sb, featT_ap)

            val_sb = [
                apool.tile([P, NT * 8], mybir.dt.float32, name=f"val_sb_{m}", tag=f"val{m}")
                for m in range(MB)
            ]
            idx_sb = [
                apool.tile([P, NT * 8], mybir.dt.uint16, name=f"idx_sb_{m}", tag=f"idx{m}")
                for m in range(MB)
            ]

            # train tiles processed in pairs: the same feat[ko,m] weights
            # feed two rhs tiles back-to-back, halving LDWEIGHTS pressure
            t = 0
            while t < NT:
                G = 2 if t + 1 < NT else 1
                tr_sb = spool.tile(
                    [P, KO2, 2, 2 * TILE_N], mm_dtype, name="tr_sb", tag="train"
                )
                nc.sync.dma_start(
                    tr_sb[:, :, :, : G * TILE_N],
                    trainT_ap[:, :, :, t * TILE_N : (t + G) * TILE_N],
                )
                for m in range(MB):
                    pss = [
                        ppool.tile([P, TILE_N], mybir.dt.float32, name="ps", tag="ps")
                        for _ in range(G)
                    ]
                    for ko in range(KO2):
                        for g in range(G):
                            nc.tensor.matmul(
                                pss[g],
                                lhsT=feat_sb[:, ko, :, m * P : (m + 1) * P],
                                rhs=tr_sb[:, ko, :, g * TILE_N : (g + 1) * TILE_N],
                                start=(ko == 0),
                                stop=(ko == KO2 - 1),
                                perf_mode=mybir.MatmulPerfMode.DoubleRow,
                            )
                    for g in range(G):
                        tt = t + g
                        vslice = val_sb[m][:, tt * 8 : (tt + 1) * 8]
                        nc.vector.max(out=vslice, in_=pss[g])
                        nc.vector.max_index(
                            out=idx_sb[m][:, tt * 8 : (tt + 1) * 8],
                            in_max=vslice,
                            in_values=pss[g],
                        )
                t += G

            ov = out_val.ap().rearrange("(mb p) c -> mb p c", p=P)
            oi = out_idx.ap().rearrange("(mb p) c -> mb p c", p=P)
            for m in range(MB):
                nc.sync.dma_start(ov[m], val_sb[m])
                nc.sync.dma_start(oi[m], idx_sb[m])

    nc.compile()
    return nc


def _get_nc():
    if "nc" not in _NC_CACHE:
        _NC_CACHE["nc"] = _build_bass()
    return _NC_CACHE["nc"]


def _make_in_maps(F, TR):
    """Quantize to fp8e4 and lay out per-core DRAM inputs."""
    TRp = np.zeros((NCORES * NCHUNK, D), np.float32)
    TRp[:N] = TR
    feat8T = np.ascontiguousarray(F.astype(ml_dtypes.float8_e4m3).T)
    TR8 = TRp.astype(ml_dtypes.float8_e4m3)
    return [
        {
            "featT": feat8T,
            "trainT": np.ascontiguousarray(TR8[c * NCHUNK : (c + 1) * NCHUNK].T),
        }
        for c in range(NCORES)
    ]


def _vote(topv, labels):
    """Reproduce the reference's softmax voting given sorted top sims.

    topv: (B', 200) fp32 descending (padded with -inf); labels (B', 200).
    """
    Bp = topv.shape[0]
    x = (topv / np.float32(T)).astype(np.float32)
    e = np.exp(x - x[:, :1], dtype=np.float32)
    s = e.sum(axis=1, keepdims=True, dtype=np.float32)
    w = (e / s).astype(np.float32)
    rows = np.broadcast_to(np.arange(Bp)[:, None], labels.shape)
    outs = []
    for k in KS:
        p = np.zeros((Bp, NUM_CLASSES), np.float32)
        np.add.at(p, (rows[:, :k], labels[:, :k]), w[:, :k])
        outs.append(p)
    return outs


def _exact_row(F, TR, LB, b):
    s = (F[b : b + 1] @ TR.T).astype(np.float32)[0]
    o = np.argsort(-s, kind="stable")[:200]
    return _vote(s[o][None].astype(np.float32), LB[o].astype(np.int64)[None])


def _combine(F, TR, LB, vals, idxs):
    NTN = NT * 8
    slot_tile = (np.arange(NTN) // 8) * TILE_N
    gcol = (
        idxs
        + slot_tile[None, None, :]
        + (np.arange(NCORES)[:, None, None] * NCHUNK)
    )
    v = vals.transpose(1, 0, 2).reshape(B, NCORES * NTN)
    g = gcol.transpose(1, 0, 2).reshape(B, NCORES * NTN)
    v = np.where(g < N, v, -np.inf).astype(np.float32)

    # approximate top-KEXACT per row
    part = np.argpartition(-v, KEXACT, axis=1)[:, :KEXACT]
    rows = np.arange(B)[:, None]
    cand_v = v[rows, part]
    cand_g = g[rows, part]

    # exact fp32 rescoring of the candidates
    exact = np.einsum(
        "bkd,bd->bk", TR[cand_g], F, optimize=True
    ).astype(np.float32)

    # sort by exact value desc, ties by train index asc (lax.top_k order)
    ordk = np.lexsort((cand_g, -exact.astype(np.float64)), axis=1)
    exact_s = np.take_along_axis(exact, ordk, axis=1)
    g_s = np.take_along_axis(cand_g, ordk, axis=1)

    topv = np.full((B, max(200, KEXACT)), -np.inf, np.float32)
    topv[:, :KEXACT] = exact_s
    labels = np.zeros((B, max(200, KEXACT)), np.int64)
    labels[:, :KEXACT] = LB[g_s].astype(np.int64)

    outs = _vote(topv, labels)

    # pathological-row triggers -> exact host recompute
    amax = cand_v.max(axis=1)
    # (i) too many candidates near the top (exact-significance window overflow)
    near = (cand_v >= (amax[:, None] - MARGIN)).sum(axis=1)
    trig_i = near >= KEXACT - 8
    # (ii) some tile's 8th approx value near the top (dropped 9th candidate)
    v8 = vals[:, :, 7::8]  # (ncores, B, NT)
    trig_ii = v8.max(axis=(0, 2)) >= amax - (MARGIN + 0.5)
    # (iii) duplicate global col among candidates (HW tie semantics)
    ss = np.sort(cand_g, axis=1)
    trig_iii = (np.diff(ss, axis=1) == 0).any(axis=1)

    ntrig = 0
    for b in np.where(trig_i | trig_ii | trig_iii)[0]:
        ob = _exact_row(F, TR, LB, b)
        for i in range(len(KS)):
            outs[i][b] = ob[i][0]
        ntrig += 1
    _combine.last_ntrig = ntrig

    return tuple(outs)


def kernel(features_rank, train_features, train_labels):
    from concourse.bass_utils import run_bass_kernel_spmd

    F = np.ascontiguousarray(np.asarray(features_rank, dtype=np.float32))
    TR = np.ascontiguousarray(np.asarray(train_features, dtype=np.float32))
    LB = np.asarray(train_labels)

    in_maps = _make_in_maps(F, TR)

    nc = _get_nc()
    res = run_bass_kernel_spmd(nc, in_maps, core_ids=list(range(NCORES)))

    vals = np.stack([np.asarray(res.results[c]["t8val"]) for c in range(NCORES)])
    idxs = np.stack(
        [np.asarray(res.results[c]["t8idx"]).astype(np.int64) for c in range(NCORES)]
    )
    return _combine(F, TR, LB, vals, idxs)


# revision 4
# speedup vs baseline: 2.0553x; 1.4189x over previous
"""TRN2 Bass kernel for nn_KnnModule (retrieval_knn).

Strategy (sharded over the 8 NeuronCores):
  - train set (100000 x 1024) is padded to 102400 rows and split into 8
    chunks of 12800; each core computes sims = features @ chunk.T
    (2048 x 12800) on the PE in fp8e4 (e4m3) with MatmulPerfMode.DoubleRow
    (two 128-row contraction halves per instruction), accumulating
    2048-wide PSUM tile-groups (4 banks, double-buffered).
  - the only on-device post-processing is a DVE TensorReduce(max) per
    (row-block, tile-group) producing the max over every 32-wide column
    group: a (2048, 400) group-max matrix per core, DMAed out.
  - host: selects the global top-48 groups per row by group-max (fp8
    precision), rescores all 48*32 = 1536 member columns exactly in fp32,
    takes the exact top-200, and reproduces the reference softmax voting
    for k in (10, 20, 100, 200).

  Why this is exact: with T=0.07 a candidate's fp32 softmax weight is
  exactly 0.0 unless its sim is within ~7.3 of the row max. fp8e4
  quantization perturbs each sim by < 8.5 (7 sigma; empirical max 6.3
  sigma, std 1.21), so every candidate that matters lives in a group
  whose group-max ranks in the global top ~18 (measured) of 3125 - far
  inside the top-48 kept. A one-sided certificate (best unrescored
  group-max must be below exact_max - 7.3 - 8.5) detects any row where
  capture could be in doubt and recomputes it exactly on the host; on
  this data it never fires.
"""

import numpy as np
import ml_dtypes

KS = (10, 20, 100, 200)
T = 0.07
NUM_CLASSES = 1000
B, N, D = 2048, 100000, 1024
NCORES = 8
NCHUNK = 12800  # per-core padded chunk (12500 real + 300 zero pad)
P = 128
KO2 = D // 256  # DoubleRow contracts 256 rows of D per matmul
GW = 32         # group width for the on-device max-reduce
GR = NCHUNK // GW  # 400 groups per core chunk
TGW = 2048      # PSUM tile-group width (4 banks); chunk = 6x2048 + 512
TG_WIDTHS = [2048] * 6 + [512]
KR = 48         # groups per row exactly rescored on host
E8 = 8.5        # fp8 sim error bound (7 sigma)
WIN = 7.3       # fp32 softmax significance window (T * 104)

_NC_CACHE = {}


def _build_bass():
    import concourse.bacc as bacc
    import concourse.mybir as mybir
    import concourse.tile as tile

    mm_dtype = mybir.dt.float8e4
    MB = B // P

    nc = bacc.Bacc(
        "TRN2",
        target_bir_lowering=False,
        debug=False,
        enable_asserts=False,
    )
    featT = nc.dram_tensor("featT", (D, B), mm_dtype, kind="ExternalInput")
    trainT = nc.dram_tensor("trainT", (D, NCHUNK), mm_dtype, kind="ExternalInput")
    out_gm = nc.dram_tensor("gmax", (B, GR), mybir.dt.float32, kind="ExternalOutput")

    # DoubleRow pairing: sim contribution of D-row d = ko*256 + half*128 + p
    featT_ap = featT.ap().rearrange("(ko two p) b -> p ko two b", p=P, two=2)
    trainT_ap = trainT.ap().rearrange("(ko two p) n -> p ko two n", p=P, two=2)
    ogm = out_gm.ap().rearrange("(mb p) g -> mb p g", p=P)

    with tile.TileContext(nc) as tc:
        with (
            tc.tile_pool(name="const", bufs=1) as cpool,
            tc.tile_pool(name="stream", bufs=2) as spool,
            tc.tile_pool(name="acc", bufs=1) as apool,
            tc.tile_pool(name="psum", bufs=2, space="PSUM") as ppool,
        ):
            feat_sb = cpool.tile([P, KO2, 2, B], mm_dtype)
            nc.sync.dma_start(feat_sb, featT_ap)

            gm_sb = [
                apool.tile([P, GR], mybir.dt.float32, name=f"gm_{m}", tag=f"gm{m}")
                for m in range(MB)
            ]

            t0 = 0
            for tg, W_ in enumerate(TG_WIDTHS):
                tr_sb = spool.tile(
                    [P, KO2, 2, TGW], mm_dtype, name="tr_sb", tag="train"
                )
                nc.sync.dma_start(
                    tr_sb[:, :, :, :W_],
                    trainT_ap[:, :, :, t0 : t0 + W_],
                )
                for m in range(MB):
                    ps = ppool.tile([P, TGW], mybir.dt.float32, name="ps", tag="ps")
                    for ko in range(KO2):
                        for g in range(W_ // 512):
                            nc.tensor.matmul(
                                ps[:, g * 512 : (g + 1) * 512],
                                lhsT=feat_sb[:, ko, :, m * P : (m + 1) * P],
                                rhs=tr_sb[:, ko, :, g * 512 : (g + 1) * 512],
                                start=(ko == 0),
                                stop=(ko == KO2 - 1),
                                perf_mode=mybir.MatmulPerfMode.DoubleRow,
                            )
                    g0 = t0 // GW
                    nc.vector.tensor_reduce(
                        out=gm_sb[m][:, g0 : g0 + W_ // GW],
                        in_=ps[:, :W_].rearrange("p (g w) -> p g w", w=GW),
                        axis=mybir.AxisListType.X,
                        op=mybir.AluOpType.max,
                    )
                    nc.sync.dma_start(
                        ogm[m][:, g0 : g0 + W_ // GW],
                        gm_sb[m][:, g0 : g0 + W_ // GW],
                    )
                t0 += W_

    nc.compile()
    return nc


def _get_nc():
    if "nc" not in _NC_CACHE:
        _NC_CACHE["nc"] = _build_bass()
    return _NC_CACHE["nc"]


def _make_in_maps(F, TR):
    """Quantize to fp8e4 and lay out per-core DRAM inputs."""
    TRp = np.zeros((NCORES * NCHUNK, D), np.float32)
    TRp[:N] = TR
    feat8T = np.ascontiguousarray(F.astype(ml_dtypes.float8_e4m3).T)
    TR8 = TRp.astype(ml_dtypes.float8_e4m3)
    return [
        {
            "featT": feat8T,
            "trainT": np.ascontiguousarray(TR8[c * NCHUNK : (c + 1) * NCHUNK].T),
        }
        for c in range(NCORES)
    ]


def _vote(topv, labels):
    """Reproduce the reference's softmax voting given sorted top sims.

    topv: (B', >=200) fp32 descending (padded with -inf); labels same shape.
    """
    Bp = topv.shape[0]
    x = (topv / np.float32(T)).astype(np.float32)
    e = np.exp(x - x[:, :1], dtype=np.float32)
    s = e.sum(axis=1, keepdims=True, dtype=np.float32)
    w = (e / s).astype(np.float32)
    rows = np.broadcast_to(np.arange(Bp)[:, None], labels.shape)
    outs = []
    for k in KS:
        p = np.zeros((Bp, NUM_CLASSES), np.float32)
        np.add.at(p, (rows[:, :k], labels[:, :k]), w[:, :k])
        outs.append(p)
    return outs


def _exact_row(F, TR, LB, b):
    s = (F[b : b + 1] @ TR.T).astype(np.float32)[0]
    o = np.argsort(-s, kind="stable")[:200]
    return _vote(s[o][None].astype(np.float32), LB[o].astype(np.int64)[None])


def _combine(F, TR, LB, gmax):
    """gmax: (B, NCORES*GR) fp8-precision group maxima."""
    NGLOB = N // GW  # 3125 real groups (N divides GW exactly)
    # global group g covers train cols [g*32, g*32+32)
    real = np.arange(NCORES * GR) * GW < N
    gm = np.where(real[None, :], gmax, -np.inf).astype(np.float32)

    rows200 = None
    topv = np.full((B, 200), -np.inf, np.float32)
    labels = np.zeros((B, 200), np.int64)
    trig = np.zeros(B, bool)

    CH = 256
    for b0 in range(0, B, CH):
        gmc = gm[b0 : b0 + CH]
        nb = gmc.shape[0]
        # top-KR groups per row by approximate group max
        part = np.argpartition(-gmc, KR, axis=1)[:, :KR]
        rows = np.arange(nb)[:, None]
        # best unrescored group-max (for the capture certificate)
        rest_max = np.copy(gmc)
        rest_max[rows, part] = -np.inf
        unresc = rest_max.max(axis=1)

        cols = (part[:, :, None] * GW + np.arange(GW)[None, None, :]).reshape(nb, KR * GW)
        valid = cols < N
        colsc = np.minimum(cols, N - 1)
        # exact fp32 rescoring of all member columns of the kept groups
        exact = np.einsum(
            "bkd,bd->bk", TR[colsc], F[b0 : b0 + CH], optimize=True
        ).astype(np.float32)
        exact[~valid] = -np.inf

        # exact top-200, ties by train index asc (lax.top_k order)
        top = np.argpartition(-exact, 200, axis=1)[:, :200]
        ev = exact[rows, top]
        eg = colsc[rows, top]
        ordk = np.lexsort((eg, -ev.astype(np.float64)), axis=1)
        ev = np.take_along_axis(ev, ordk, axis=1)
        eg = np.take_along_axis(eg, ordk, axis=1)
        topv[b0 : b0 + CH] = ev
        labels[b0 : b0 + CH] = LB[eg].astype(np.int64)

        # capture certificate: any unrescored group could hide a candidate
        # only if its (approx) max is within WIN + E8 of the exact row max
        m_e = ev[:, 0]
        trig[b0 : b0 + CH] = unresc >= m_e - (WIN + E8)

    outs = _vote(topv, labels)

    ntrig = 0
    for b in np.where(trig)[0]:
        ob = _exact_row(F, TR, LB, b)
        for i in range(len(KS)):
            outs[i][b] = ob[i][0]
        ntrig += 1
    _combine.last_ntrig = ntrig

    return tuple(outs)


def kernel(features_rank, train_features, train_labels):
    from concourse.bass_utils import run_bass_kernel_spmd

    F = np.ascontiguousarray(np.asarray(features_rank, dtype=np.float32))
    TR = np.ascontiguousarray(np.asarray(train_features, dtype=np.float32))
    LB = np.asarray(train_labels)

    in_maps = _make_in_maps(F, TR)

    nc = _get_nc()
    res = run_bass_kernel_spmd(nc, in_maps, core_ids=list(range(NCORES)))

    gmax = np.concatenate(
        [np.asarray(res.results[c]["gmax"]) for c in range(NCORES)], axis=1
    )
    return _combine(F, TR, LB, gmax)


# revision 10
# speedup vs baseline: 2.0754x; 1.0098x over previous
"""TRN2 Bass kernel for nn_KnnModule (retrieval_knn).

Strategy (sharded over the 8 NeuronCores):
  - train set (100000 x 1024) is padded to 102400 rows and split into 8
    chunks of 12800; each core computes sims = features @ chunk.T
    (2048 x 12800) on the PE in fp8e4 (e4m3) with MatmulPerfMode.DoubleRow
    (two 128-row contraction halves per instruction), accumulating
    2048-wide PSUM tile-groups (4 banks, double-buffered).
  - the only on-device post-processing is a DVE TensorReduce(max) per
    (row-block, tile-group) producing the max over every 32-wide column
    group: a (2048, 400) group-max matrix per core, DMAed out.
  - host: selects the global top-48 groups per row by group-max (fp8
    precision), rescores all 48*32 = 1536 member columns exactly in fp32,
    takes the exact top-200, and reproduces the reference softmax voting
    for k in (10, 20, 100, 200).

  Why this is exact: with T=0.07 a candidate's fp32 softmax weight is
  exactly 0.0 unless its sim is within ~7.3 of the row max. fp8e4
  quantization perturbs each sim by < 8.5 (7 sigma; empirical max 6.3
  sigma, std 1.21), so every candidate that matters lives in a group
  whose group-max ranks in the global top ~18 (measured) of 3125 - far
  inside the top-48 kept. A one-sided certificate (best unrescored
  group-max must be below exact_max - 7.3 - 8.5) detects any row where
  capture could be in doubt and recomputes it exactly on the host; on
  this data it never fires.
"""

import numpy as np
import ml_dtypes

KS = (10, 20, 100, 200)
T = 0.07
NUM_CLASSES = 1000
B, N, D = 2048, 100000, 1024
NCORES = 8
NCHUNK = 12800  # per-core padded chunk (12500 real + 300 zero pad)
P = 128
KO2 = D // 256  # DoubleRow contracts 256 rows of D per matmul
GW = 32         # group width for the on-device max-reduce
GR = NCHUNK // GW  # 400 groups per core chunk
TGW = 2048      # PSUM tile-group width (4 banks); chunk = 512 + 6x2048
# the small tail group goes FIRST: its train DMA is 4x smaller, so the PE
# starts ~15us earlier while the first full-size group streams in behind it
TG_WIDTHS = [512] + [2048] * 6
KR = 48         # groups per row exactly rescored on host
E8 = 8.5        # fp8 sim error bound (7 sigma)
WIN = 7.3       # fp32 softmax significance window (T * 104)

_NC_CACHE = {}


def _build_bass():
    import concourse.bacc as bacc
    import concourse.mybir as mybir
    import concourse.tile as tile

    mm_dtype = mybir.dt.float8e4
    MB = B // P

    nc = bacc.Bacc(
        "TRN2",
        target_bir_lowering=False,
        debug=False,
        enable_asserts=False,
    )
    featT = nc.dram_tensor("featT", (D, B), mm_dtype, kind="ExternalInput")
    trainT = nc.dram_tensor("trainT", (D, NCHUNK), mm_dtype, kind="ExternalInput")
    out_gm = nc.dram_tensor("gmax", (B, GR), mybir.dt.float32, kind="ExternalOutput")

    # DoubleRow pairing: sim contribution of D-row d = ko*256 + half*128 + p
    featT_ap = featT.ap().rearrange("(ko two p) b -> p ko two b", p=P, two=2)
    trainT_ap = trainT.ap().rearrange("(ko two p) n -> p ko two n", p=P, two=2)
    ogm = out_gm.ap().rearrange("(mb p) g -> mb p g", p=P)

    with tile.TileContext(nc) as tc:
        with (
            tc.tile_pool(name="const", bufs=1) as cpool,
            tc.tile_pool(name="stream", bufs=2) as spool,
            tc.tile_pool(name="acc", bufs=1) as apool,
            tc.tile_pool(name="psum", bufs=2, space="PSUM") as ppool,
        ):
            # one tile per contraction group so the first matmul only waits
            # on a quarter of the features transfer
            feat_sb = [
                cpool.tile([P, 2, B], mm_dtype, name=f"feat_{ko}", tag=f"feat{ko}")
                for ko in range(KO2)
            ]
            for ko in range(KO2):
                nc.sync.dma_start(feat_sb[ko], featT_ap[:, ko])

            gm_sb = [
                apool.tile([P, GR], mybir.dt.float32, name=f"gm_{m}", tag=f"gm{m}")
                for m in range(MB)
            ]

            t0 = 0
            for tg, W_ in enumerate(TG_WIDTHS):
                tr_sb = spool.tile(
                    [P, KO2, 2, TGW], mm_dtype, name="tr_sb", tag="train"
                )
                nc.sync.dma_start(
                    tr_sb[:, :, :, :W_],
                    trainT_ap[:, :, :, t0 : t0 + W_],
                )
                for m in range(MB):
                    ps = ppool.tile([P, TGW], mybir.dt.float32, name="ps", tag="ps")
                    for ko in range(KO2):
                        for g in range(W_ // 512):
                            nc.tensor.matmul(
                                ps[:, g * 512 : (g + 1) * 512],
                                lhsT=feat_sb[ko][:, :, m * P : (m + 1) * P],
                                rhs=tr_sb[:, ko, :, g * 512 : (g + 1) * 512],
                                start=(ko == 0),
                                stop=(ko == KO2 - 1),
                                perf_mode=mybir.MatmulPerfMode.DoubleRow,
                            )
                    g0 = t0 // GW
                    nc.vector.tensor_reduce(
                        out=gm_sb[m][:, g0 : g0 + W_ // GW],
                        in_=ps[:, :W_].rearrange("p (g w) -> p g w", w=GW),
                        axis=mybir.AxisListType.X,
                        op=mybir.AluOpType.max,
                    )
                    nc.sync.dma_start(
                        ogm[m][:, g0 : g0 + W_ // GW],
                        gm_sb[m][:, g0 : g0 + W_ // GW],
                    )
                t0 += W_

    nc.compile()
    return nc


def _get_nc():
    if "nc" not in _NC_CACHE:
        _NC_CACHE["nc"] = _build_bass()
    return _NC_CACHE["nc"]


def _make_in_maps(F, TR):
    """Quantize to fp8e4 and lay out per-core DRAM inputs."""
    TRp = np.zeros((NCORES * NCHUNK, D), np.float32)
    TRp[:N] = TR
    feat8T = np.ascontiguousarray(F.astype(ml_dtypes.float8_e4m3).T)
    TR8 = TRp.astype(ml_dtypes.float8_e4m3)
    return [
        {
            "featT": feat8T,
            "trainT": np.ascontiguousarray(TR8[c * NCHUNK : (c + 1) * NCHUNK].T),
        }
        for c in range(NCORES)
    ]


def _vote(topv, labels):
    """Reproduce the reference's softmax voting given sorted top sims.

    topv: (B', >=200) fp32 descending (padded with -inf); labels same shape.
    """
    Bp = topv.shape[0]
    x = (topv / np.float32(T)).astype(np.float32)
    e = np.exp(x - x[:, :1], dtype=np.float32)
    s = e.sum(axis=1, keepdims=True, dtype=np.float32)
    w = (e / s).astype(np.float32)
    rows = np.broadcast_to(np.arange(Bp)[:, None], labels.shape)
    outs = []
    for k in KS:
        p = np.zeros((Bp, NUM_CLASSES), np.float32)
        np.add.at(p, (rows[:, :k], labels[:, :k]), w[:, :k])
        outs.append(p)
    return outs


def _exact_row(F, TR, LB, b):
    s = (F[b : b + 1] @ TR.T).astype(np.float32)[0]
    o = np.argsort(-s, kind="stable")[:200]
    return _vote(s[o][None].astype(np.float32), LB[o].astype(np.int64)[None])


def _combine(F, TR, LB, gmax):
    """gmax: (B, NCORES*GR) fp8-precision group maxima."""
    NGLOB = N // GW  # 3125 real groups (N divides GW exactly)
    # global group g covers train cols [g*32, g*32+32)
    real = np.arange(NCORES * GR) * GW < N
    gm = np.where(real[None, :], gmax, -np.inf).astype(np.float32)

    rows200 = None
    topv = np.full((B, 200), -np.inf, np.float32)
    labels = np.zeros((B, 200), np.int64)
    trig = np.zeros(B, bool)

    CH = 256
    for b0 in range(0, B, CH):
        gmc = gm[b0 : b0 + CH]
        nb = gmc.shape[0]
        # top-KR groups per row by approximate group max
        part = np.argpartition(-gmc, KR, axis=1)[:, :KR]
        rows = np.arange(nb)[:, None]
        # best unrescored group-max (for the capture certificate)
        rest_max = np.copy(gmc)
        rest_max[rows, part] = -np.inf
        unresc = rest_max.max(axis=1)

        cols = (part[:, :, None] * GW + np.arange(GW)[None, None, :]).reshape(nb, KR * GW)
        valid = cols < N
        colsc = np.minimum(cols, N - 1)
        # exact fp32 rescoring of all member columns of the kept groups
        exact = np.einsum(
            "bkd,bd->bk", TR[colsc], F[b0 : b0 + CH], optimize=True
        ).astype(np.float32)
        exact[~valid] = -np.inf

        # exact top-200, ties by train index asc (lax.top_k order)
        top = np.argpartition(-exact, 200, axis=1)[:, :200]
        ev = exact[rows, top]
        eg = colsc[rows, top]
        ordk = np.lexsort((eg, -ev.astype(np.float64)), axis=1)
        ev = np.take_along_axis(ev, ordk, axis=1)
        eg = np.take_along_axis(eg, ordk, axis=1)
        topv[b0 : b0 + CH] = ev
        labels[b0 : b0 + CH] = LB[eg].astype(np.int64)

        # capture certificate: any unrescored group could hide a candidate
        # only if its (approx) max is within WIN + E8 of the exact row max
        m_e = ev[:, 0]
        trig[b0 : b0 + CH] = unresc >= m_e - (WIN + E8)

    outs = _vote(topv, labels)

    ntrig = 0
    for b in np.where(trig)[0]:
        ob = _exact_row(F, TR, LB, b)
        for i in range(len(KS)):
            outs[i][b] = ob[i][0]
        ntrig += 1
    _combine.last_ntrig = ntrig

    return tuple(outs)


def kernel(features_rank, train_features, train_labels):
    from concourse.bass_utils import run_bass_kernel_spmd

    F = np.ascontiguousarray(np.asarray(features_rank, dtype=np.float32))
    TR = np.ascontiguousarray(np.asarray(train_features, dtype=np.float32))
    LB = np.asarray(train_labels)

    in_maps = _make_in_maps(F, TR)

    nc = _get_nc()
    res = run_bass_kernel_spmd(nc, in_maps, core_ids=list(range(NCORES)))

    gmax = np.concatenate(
        [np.asarray(res.results[c]["gmax"]) for c in range(NCORES)], axis=1
    )
    return _combine(F, TR, LB, gmax)


# revision 19
# speedup vs baseline: 2.1139x; 1.0185x over previous
"""TRN2 Bass kernel for nn_KnnModule (retrieval_knn).

Strategy (sharded over the 8 NeuronCores):
  - train set (100000 x 1024) is padded to 102400 rows and split into 8
    chunks of 12800; each core computes sims = features @ chunk.T
    (2048 x 12800) on the PE in fp8e4 (e4m3) with MatmulPerfMode.DoubleRow
    (two 128-row contraction halves per instruction), accumulating
    2048-wide PSUM tile-groups (4 banks, double-buffered).
  - the only on-device post-processing is a DVE TensorReduce(max) per
    (row-block, tile-group) producing the max over every 32-wide column
    group: a (2048, 400) group-max matrix per core, DMAed out.
  - host: selects the global top-48 groups per row by group-max (fp8
    precision), rescores all 48*32 = 1536 member columns exactly in fp32,
    takes the exact top-200, and reproduces the reference softmax voting
    for k in (10, 20, 100, 200).

  Why this is exact: with T=0.07 a candidate's fp32 softmax weight is
  exactly 0.0 unless its sim is within ~7.3 of the row max. fp8e4
  quantization perturbs each sim by < 8.5 (7 sigma; empirical max 6.3
  sigma, std 1.21), so every candidate that matters lives in a group
  whose group-max ranks in the global top ~18 (measured) of 3125 - far
  inside the top-48 kept. A one-sided certificate (best unrescored
  group-max must be below exact_max - 7.3 - 8.5) detects any row where
  capture could be in doubt and recomputes it exactly on the host; on
  this data it never fires.
"""

import numpy as np
import ml_dtypes

KS = (10, 20, 100, 200)
T = 0.07
NUM_CLASSES = 1000
B, N, D = 2048, 100000, 1024
NCORES = 8
NCHUNK = 12512  # per-core padded chunk (12500 real + 12 zero pad)
P = 128
KO2 = D // 256  # DoubleRow contracts 256 rows of D per matmul
GW = 32         # group width for the on-device max-reduce
GR = NCHUNK // GW  # 391 groups per core chunk
TGW = 2048      # PSUM tile-group width (4 banks); chunk = 6x2048 + 224
TG_WIDTHS = [2048] * 6 + [224]
KR = 48         # groups per row exactly rescored on host
E8 = 8.5        # fp8 sim error bound (7 sigma)
WIN = 7.3       # fp32 softmax significance window (T * 104)

_NC_CACHE = {}


def _build_bass(split_startup=True):
    import concourse.bacc as bacc
    import concourse.mybir as mybir
    import concourse.tile as tile

    mm_dtype = mybir.dt.float8e4
    MB = B // P

    nc = bacc.Bacc(
        "TRN2",
        target_bir_lowering=False,
        debug=False,
        enable_asserts=False,
    )
    featT = nc.dram_tensor("featT", (D, B), mm_dtype, kind="ExternalInput")
    trainT = nc.dram_tensor("trainT", (D, NCHUNK), mm_dtype, kind="ExternalInput")
    out_gm = nc.dram_tensor("gmax", (B, GR), mybir.dt.float32, kind="ExternalOutput")

    # DoubleRow pairing: sim contribution of D-row d = ko*256 + half*128 + p
    featT_ap = featT.ap().rearrange("(ko two p) b -> p ko two b", p=P, two=2)
    trainT_ap = trainT.ap().rearrange("(ko two p) n -> p ko two n", p=P, two=2)
    ogm = out_gm.ap().rearrange("(mb p) g -> mb p g", p=P)

    with tile.TileContext(nc) as tc:
        with (
            tc.tile_pool(name="const", bufs=1) as cpool,
            tc.tile_pool(name="stream", bufs=2) as spool,
            tc.tile_pool(name="acc", bufs=1) as apool,
            tc.tile_pool(name="psum", bufs=2, space="PSUM") as ppool,
        ):
            # the first train group and the features are both on the matmul
            # critical path at startup: issue the first group's transfer
            # before the features, and split both per contraction group so
            # the first matmul only waits on a quarter of each
            feat_sb = [
                cpool.tile([P, 2, B], mm_dtype, name=f"feat_{ko}", tag=f"feat{ko}")
                for ko in range(KO2)
            ]
            tr0_sb = None
            if split_startup:
                tr0_sb = spool.tile(
                    [P, KO2, 2, TGW], mm_dtype, name="tr_sb", tag="train"
                )
                nc.sync.dma_start(tr0_sb[:, 0], trainT_ap[:, 0, :, :TGW])
                nc.sync.dma_start(feat_sb[0], featT_ap[:, 0])
                for ko in range(1, KO2):
                    nc.sync.dma_start(tr0_sb[:, ko], trainT_ap[:, ko, :, :TGW])
                    nc.sync.dma_start(feat_sb[ko], featT_ap[:, ko])
            else:
                for ko in range(KO2):
                    nc.sync.dma_start(feat_sb[ko], featT_ap[:, ko])

            gm_sb = [
                apool.tile([P, GR], mybir.dt.float32, name=f"gm_{m}", tag=f"gm{m}")
                for m in range(MB)
            ]

            t0 = 0
            for tg, W_ in enumerate(TG_WIDTHS):
                if tg == 0 and tr0_sb is not None:
                    tr_sb = tr0_sb
                else:
                    tr_sb = spool.tile(
                        [P, KO2, 2, TGW], mm_dtype, name="tr_sb", tag="train"
                    )
                    nc.sync.dma_start(
                        tr_sb[:, :, :, :W_],
                        trainT_ap[:, :, :, t0 : t0 + W_],
                    )
                for m in range(MB):
                    ps = ppool.tile([P, TGW], mybir.dt.float32, name="ps", tag="ps")
                    for ko in range(KO2):
                        for g0_ in range(0, W_, 512):
                            g1_ = min(g0_ + 512, W_)
                            nc.tensor.matmul(
                                ps[:, g0_:g1_],
                                lhsT=feat_sb[ko][:, :, m * P : (m + 1) * P],
                                rhs=tr_sb[:, ko, :, g0_:g1_],
                                start=(ko == 0),
                                stop=(ko == KO2 - 1),
                                perf_mode=mybir.MatmulPerfMode.DoubleRow,
                            )
                    g0 = t0 // GW
                    nc.vector.tensor_reduce(
                        out=gm_sb[m][:, g0 : g0 + W_ // GW],
                        in_=ps[:, :W_].rearrange("p (g w) -> p g w", w=GW),
                        axis=mybir.AxisListType.X,
                        op=mybir.AluOpType.max,
                    )
                    nc.sync.dma_start(
                        ogm[m][:, g0 : g0 + W_ // GW],
                        gm_sb[m][:, g0 : g0 + W_ // GW],
                    )
                t0 += W_

    nc.compile()
    return nc


def _get_nc():
    if "nc" not in _NC_CACHE:
        _NC_CACHE["nc"] = _build_bass()
    return _NC_CACHE["nc"]


_VARIANTS = {
    "split": lambda: _build_bass(split_startup=True),
    "plain": lambda: _build_bass(split_startup=False),
}


def _make_in_maps(F, TR):
    """Quantize to fp8e4 and lay out per-core DRAM inputs."""
    TRp = np.zeros((NCORES * NCHUNK, D), np.float32)
    TRp[:N] = TR
    feat8T = np.ascontiguousarray(F.astype(ml_dtypes.float8_e4m3).T)
    TR8 = TRp.astype(ml_dtypes.float8_e4m3)
    return [
        {
            "featT": feat8T,
            "trainT": np.ascontiguousarray(TR8[c * NCHUNK : (c + 1) * NCHUNK].T),
        }
        for c in range(NCORES)
    ]


def _vote(topv, labels):
    """Reproduce the reference's softmax voting given sorted top sims.

    topv: (B', >=200) fp32 descending (padded with -inf); labels same shape.
    """
    Bp = topv.shape[0]
    x = (topv / np.float32(T)).astype(np.float32)
    e = np.exp(x - x[:, :1], dtype=np.float32)
    s = e.sum(axis=1, keepdims=True, dtype=np.float32)
    w = (e / s).astype(np.float32)
    rows = np.broadcast_to(np.arange(Bp)[:, None], labels.shape)
    outs = []
    for k in KS:
        p = np.zeros((Bp, NUM_CLASSES), np.float32)
        np.add.at(p, (rows[:, :k], labels[:, :k]), w[:, :k])
        outs.append(p)
    return outs


def _exact_row(F, TR, LB, b):
    s = (F[b : b + 1] @ TR.T).astype(np.float32)[0]
    o = np.argsort(-s, kind="stable")[:200]
    return _vote(s[o][None].astype(np.float32), LB[o].astype(np.int64)[None])


def _combine(F, TR, LB, gmax):
    """gmax: (B, NCORES*GR) fp8-precision group maxima."""
    NGLOB = N // GW  # 3125 real groups (N divides GW exactly)
    # global group g covers train cols [g*32, g*32+32)
    real = np.arange(NCORES * GR) * GW < N
    gm = np.where(real[None, :], gmax, -np.inf).astype(np.float32)

    rows200 = None
    topv = np.full((B, 200), -np.inf, np.float32)
    labels = np.zeros((B, 200), np.int64)
    trig = np.zeros(B, bool)

    CH = 256
    for b0 in range(0, B, CH):
        gmc = gm[b0 : b0 + CH]
        nb = gmc.shape[0]
        # top-KR groups per row by approximate group max
        part = np.argpartition(-gmc, KR, axis=1)[:, :KR]
        rows = np.arange(nb)[:, None]
        # best unrescored group-max (for the capture certificate)
        rest_max = np.copy(gmc)
        rest_max[rows, part] = -np.inf
        unresc = rest_max.max(axis=1)

        cols = (part[:, :, None] * GW + np.arange(GW)[None, None, :]).reshape(nb, KR * GW)
        valid = cols < N
        colsc = np.minimum(cols, N - 1)
        # exact fp32 rescoring of all member columns of the kept groups
        exact = np.einsum(
            "bkd,bd->bk", TR[colsc], F[b0 : b0 + CH], optimize=True
        ).astype(np.float32)
        exact[~valid] = -np.inf

        # exact top-200, ties by train index asc (lax.top_k order)
        top = np.argpartition(-exact, 200, axis=1)[:, :200]
        ev = exact[rows, top]
        eg = colsc[rows, top]
        ordk = np.lexsort((eg, -ev.astype(np.float64)), axis=1)
        ev = np.take_along_axis(ev, ordk, axis=1)
        eg = np.take_along_axis(eg, ordk, axis=1)
        topv[b0 : b0 + CH] = ev
        labels[b0 : b0 + CH] = LB[eg].astype(np.int64)

        # capture certificate: any unrescored group could hide a candidate
        # only if its (approx) max is within WIN + E8 of the exact row max
        m_e = ev[:, 0]
        trig[b0 : b0 + CH] = unresc >= m_e - (WIN + E8)

    outs = _vote(topv, labels)

    ntrig = 0
    for b in np.where(trig)[0]:
        ob = _exact_row(F, TR, LB, b)
        for i in range(len(KS)):
            outs[i][b] = ob[i][0]
        ntrig += 1
    _combine.last_ntrig = ntrig

    return tuple(outs)


def kernel(features_rank, train_features, train_labels):
    from concourse.bass_utils import run_bass_kernel_spmd

    F = np.ascontiguousarray(np.asarray(features_rank, dtype=np.float32))
    TR = np.ascontiguousarray(np.asarray(train_features, dtype=np.float32))
    LB = np.asarray(train_labels)

    in_maps = _make_in_maps(F, TR)

    nc = _get_nc()
    res = run_bass_kernel_spmd(nc, in_maps, core_ids=list(range(NCORES)))

    gmax = np.concatenate(
        [np.asarray(res.results[c]["gmax"]) for c in range(NCORES)], axis=1
    )
    return _combine(F, TR, LB, gmax)


# revision 20
# speedup vs baseline: 2.1167x; 1.0013x over previous
"""TRN2 Bass kernel for nn_KnnModule (retrieval_knn).

Strategy (sharded over the 8 NeuronCores):
  - train set (100000 x 1024) is padded to 102400 rows and split into 8
    chunks of 12800; each core computes sims = features @ chunk.T
    (2048 x 12800) on the PE in fp8e4 (e4m3) with MatmulPerfMode.DoubleRow
    (two 128-row contraction halves per instruction), accumulating
    2048-wide PSUM tile-groups (4 banks, double-buffered).
  - the only on-device post-processing is a DVE TensorReduce(max) per
    (row-block, tile-group) producing the max over every 32-wide column
    group: a (2048, 400) group-max matrix per core, DMAed out.
  - host: selects the global top-48 groups per row by group-max (fp8
    precision), rescores all 48*32 = 1536 member columns exactly in fp32,
    takes the exact top-200, and reproduces the reference softmax voting
    for k in (10, 20, 100, 200).

  Why this is exact: with T=0.07 a candidate's fp32 softmax weight is
  exactly 0.0 unless its sim is within ~7.3 of the row max. fp8e4
  quantization perturbs each sim by < 8.5 (7 sigma; empirical max 6.3
  sigma, std 1.21), so every candidate that matters lives in a group
  whose group-max ranks in the global top ~18 (measured) of 3125 - far
  inside the top-48 kept. A one-sided certificate (best unrescored
  group-max must be below exact_max - 7.3 - 8.5) detects any row where
  capture could be in doubt and recomputes it exactly on the host; on
  this data it never fires.
"""

import numpy as np
import ml_dtypes

KS = (10, 20, 100, 200)
T = 0.07
NUM_CLASSES = 1000
B, N, D = 2048, 100000, 1024
NCORES = 8
NCHUNK = 12512  # per-core padded chunk (12500 real + 12 zero pad)
P = 128
KO2 = D // 256  # DoubleRow contracts 256 rows of D per matmul
GW = 32         # group width for the on-device max-reduce
GR = NCHUNK // GW  # 391 groups per core chunk
TGW = 2048      # PSUM tile-group width (4 banks); chunk = 6x2048 + 224
TG_WIDTHS = [2048] * 6 + [224]
KR = 48         # groups per row exactly rescored on host
E8 = 8.5        # fp8 sim error bound (7 sigma)
WIN = 7.3       # fp32 softmax significance window (T * 104)

_NC_CACHE = {}


def _build_bass(split_startup=True):
    import concourse.bacc as bacc
    import concourse.mybir as mybir
    import concourse.tile as tile

    mm_dtype = mybir.dt.float8e4
    MB = B // P

    nc = bacc.Bacc(
        "TRN2",
        target_bir_lowering=False,
        debug=False,
        enable_asserts=False,
    )
    featT = nc.dram_tensor("featT", (D, B), mm_dtype, kind="ExternalInput")
    trainT = nc.dram_tensor("trainT", (D, NCHUNK), mm_dtype, kind="ExternalInput")
    out_gm = nc.dram_tensor("gmax", (B, GR), mybir.dt.float32, kind="ExternalOutput")

    # DoubleRow pairing: sim contribution of D-row d = ko*256 + half*128 + p
    featT_ap = featT.ap().rearrange("(ko two p) b -> p ko two b", p=P, two=2)
    trainT_ap = trainT.ap().rearrange("(ko two p) n -> p ko two n", p=P, two=2)
    ogm = out_gm.ap().rearrange("(mb p) g -> mb p g", p=P)

    with tile.TileContext(nc) as tc:
        with (
            tc.tile_pool(name="const", bufs=1) as cpool,
            tc.tile_pool(name="stream", bufs=2) as spool,
            tc.tile_pool(name="acc", bufs=1) as apool,
            tc.tile_pool(name="psum", bufs=2, space="PSUM") as ppool,
        ):
            # the first train group and the features are both on the matmul
            # critical path at startup: issue the first group's transfer
            # before the features, and split both per contraction group so
            # the first matmul only waits on a quarter of each
            feat_sb = [
                cpool.tile([P, 2, B], mm_dtype, name=f"feat_{ko}", tag=f"feat{ko}")
                for ko in range(KO2)
            ]
            tr0_sb = None
            if split_startup:
                tr0_sb = spool.tile(
                    [P, KO2, 2, TGW], mm_dtype, name="tr_sb", tag="train"
                )
                nc.sync.dma_start(tr0_sb[:, 0], trainT_ap[:, 0, :, :TGW])
                nc.sync.dma_start(feat_sb[0], featT_ap[:, 0])
                for ko in range(1, KO2):
                    nc.sync.dma_start(tr0_sb[:, ko], trainT_ap[:, ko, :, :TGW])
                    nc.sync.dma_start(feat_sb[ko], featT_ap[:, ko])
            else:
                for ko in range(KO2):
                    nc.sync.dma_start(feat_sb[ko], featT_ap[:, ko])

            gm_sb = [
                apool.tile([P, GR], mybir.dt.float32, name=f"gm_{m}", tag=f"gm{m}")
                for m in range(MB)
            ]

            t0 = 0
            for tg, W_ in enumerate(TG_WIDTHS):
                if tg == 0 and tr0_sb is not None:
                    tr_sb = tr0_sb
                else:
                    tr_sb = spool.tile(
                        [P, KO2, 2, TGW], mm_dtype, name="tr_sb", tag="train"
                    )
                    nc.sync.dma_start(
                        tr_sb[:, :, :, :W_],
                        trainT_ap[:, :, :, t0 : t0 + W_],
                    )
                for m in range(MB):
                    ps = ppool.tile([P, TGW], mybir.dt.float32, name="ps", tag="ps")
                    for ko in range(KO2):
                        for g0_ in range(0, W_, 512):
                            g1_ = min(g0_ + 512, W_)
                            bi = nc.tensor.matmul(
                                ps[:, g0_:g1_],
                                lhsT=feat_sb[ko][:, :, m * P : (m + 1) * P],
                                rhs=tr_sb[:, ko, :, g0_:g1_],
                                start=(ko == 0),
                                stop=(ko == KO2 - 1),
                                perf_mode=mybir.MatmulPerfMode.DoubleRow,
                            )
                            if g0_ > 0:
                                # weights are already in the PE array from
                                # this (m, ko) group's first matmul
                                bi.ins.ldweights = False
                    g0 = t0 // GW
                    nc.vector.tensor_reduce(
                        out=gm_sb[m][:, g0 : g0 + W_ // GW],
                        in_=ps[:, :W_].rearrange("p (g w) -> p g w", w=GW),
                        axis=mybir.AxisListType.X,
                        op=mybir.AluOpType.max,
                    )
                    nc.sync.dma_start(
                        ogm[m][:, g0 : g0 + W_ // GW],
                        gm_sb[m][:, g0 : g0 + W_ // GW],
                    )
                t0 += W_

    nc.compile()
    return nc


def _get_nc():
    if "nc" not in _NC_CACHE:
        _NC_CACHE["nc"] = _build_bass()
    return _NC_CACHE["nc"]


_VARIANTS = {
    "split": lambda: _build_bass(split_startup=True),
    "plain": lambda: _build_bass(split_startup=False),
}


def _make_in_maps(F, TR):
    """Quantize to fp8e4 and lay out per-core DRAM inputs."""
    TRp = np.zeros((NCORES * NCHUNK, D), np.float32)
    TRp[:N] = TR
    feat8T = np.ascontiguousarray(F.astype(ml_dtypes.float8_e4m3).T)
    TR8 = TRp.astype(ml_dtypes.float8_e4m3)
    return [
        {
            "featT": feat8T,
            "trainT": np.ascontiguousarray(TR8[c * NCHUNK : (c + 1) * NCHUNK].T),
        }
        for c in range(NCORES)
    ]


def _vote(topv, labels):
    """Reproduce the reference's softmax voting given sorted top sims.

    topv: (B', >=200) fp32 descending (padded with -inf); labels same shape.
    """
    Bp = topv.shape[0]
    x = (topv / np.float32(T)).astype(np.float32)
    e = np.exp(x - x[:, :1], dtype=np.float32)
    s = e.sum(axis=1, keepdims=True, dtype=np.float32)
    w = (e / s).astype(np.float32)
    rows = np.broadcast_to(np.arange(Bp)[:, None], labels.shape)
    outs = []
    for k in KS:
        p = np.zeros((Bp, NUM_CLASSES), np.float32)
        np.add.at(p, (rows[:, :k], labels[:, :k]), w[:, :k])
        outs.append(p)
    return outs


def _exact_row(F, TR, LB, b):
    s = (F[b : b + 1] @ TR.T).astype(np.float32)[0]
    o = np.argsort(-s, kind="stable")[:200]
    return _vote(s[o][None].astype(np.float32), LB[o].astype(np.int64)[None])


def _combine(F, TR, LB, gmax):
    """gmax: (B, NCORES*GR) fp8-precision group maxima."""
    NGLOB = N // GW  # 3125 real groups (N divides GW exactly)
    # global group g covers train cols [g*32, g*32+32)
    real = np.arange(NCORES * GR) * GW < N
    gm = np.where(real[None, :], gmax, -np.inf).astype(np.float32)

    rows200 = None
    topv = np.full((B, 200), -np.inf, np.float32)
    labels = np.zeros((B, 200), np.int64)
    trig = np.zeros(B, bool)

    CH = 256
    for b0 in range(0, B, CH):
        gmc = gm[b0 : b0 + CH]
        nb = gmc.shape[0]
        # top-KR groups per row by approximate group max
        part = np.argpartition(-gmc, KR, axis=1)[:, :KR]
        rows = np.arange(nb)[:, None]
        # best unrescored group-max (for the capture certificate)
        rest_max = np.copy(gmc)
        rest_max[rows, part] = -np.inf
        unresc = rest_max.max(axis=1)

        cols = (part[:, :, None] * GW + np.arange(GW)[None, None, :]).reshape(nb, KR * GW)
        valid = cols < N
        colsc = np.minimum(cols, N - 1)
        # exact fp32 rescoring of all member columns of the kept groups
        exact = np.einsum(
            "bkd,bd->bk", TR[colsc], F[b0 : b0 + CH], optimize=True
        ).astype(np.float32)
        exact[~valid] = -np.inf

        # exact top-200, ties by train index asc (lax.top_k order)
        top = np.argpartition(-exact, 200, axis=1)[:, :200]
        ev = exact[rows, top]
        eg = colsc[rows, top]
        ordk = np.lexsort((eg, -ev.astype(np.float64)), axis=1)
        ev = np.take_along_axis(ev, ordk, axis=1)
        eg = np.take_along_axis(eg, ordk, axis=1)
        topv[b0 : b0 + CH] = ev
        labels[b0 : b0 + CH] = LB[eg].astype(np.int64)

        # capture certificate: any unrescored group could hide a candidate
        # only if its (approx) max is within WIN + E8 of the exact row max
        m_e = ev[:, 0]
        trig[b0 : b0 + CH] = unresc >= m_e - (WIN + E8)

    outs = _vote(topv, labels)

    ntrig = 0
    for b in np.where(trig)[0]:
        ob = _exact_row(F, TR, LB, b)
        for i in range(len(KS)):
            outs[i][b] = ob[i][0]
        ntrig += 1
    _combine.last_ntrig = ntrig

    return tuple(outs)


def kernel(features_rank, train_features, train_labels):
    from concourse.bass_utils import run_bass_kernel_spmd

    F = np.ascontiguousarray(np.asarray(features_rank, dtype=np.float32))
    TR = np.ascontiguousarray(np.asarray(train_features, dtype=np.float32))
    LB = np.asarray(train_labels)

    in_maps = _make_in_maps(F, TR)

    nc = _get_nc()
    res = run_bass_kernel_spmd(nc, in_maps, core_ids=list(range(NCORES)))

    gmax = np.concatenate(
        [np.asarray(res.results[c]["gmax"]) for c in range(NCORES)], axis=1
    )
    return _combine(F, TR, LB, gmax)
